# revision 1
# baseline (speedup 1.0000x reference)
"""Trainium2 Bass kernel for nn_CMLITargetLoss (CMLI target loss).

Data parallel: batch 128 -> 16 samples per core x 8 cores. Inputs are cast
fp32->bf16 during DMA (SWDGE); all accumulation is fp32.

Per sample:
  - target^T via PE matmul-identity transposes (bf16 -> fp32 psum), copied
    back to SBUF as bf16 in [128,416] batches
  - rsq[n] = sum_d target^2 via scalar_tensor_tensor accum (per-row sums),
    then tiny PE matmuls to turn the columns into a [1,197] psum row
  - r = sqrt(rsq) on ACT, rinv = 1/r on DVE, rsqC = rsq + C on ACT;
    broadcast rows to [128,197] via ones-matmul (pair-stacked)
  - G[t,n] = text . target via PE matmul (bf16), two samples stacked
  - s = G * rinv; m = rowmax(s); mask = (s >= m); v = rsqC - 2G;
    vsel = max(mask*v) - C  =>  tok_sq = ||text_t||^2 + rsq[n*] - 2 G[t,n*]
  - image loss: diff = image - target (DVE/POOL), per-row sum of squares via
    ACT Square+accum / DVE stt accum into a [128,32] column buffer
Host combines the 8 cores' partial sums in float64.

Outputs per core: out_cols [128,4] f32: col0 masked tok_sq partials,
col1 keep partials, col2 rows 0:16 cls partials, col3 image-loss partials.
"""

import numpy as np

B, T, N, D = 128, 64, 197, 768
NCORES = 8
BL = B // NCORES  # 16 samples per core
PAIRS = BL // 2
C_OFF = float(2.0**20)
CW = 208  # transposed-target column block (128 + 80)

_CACHE = {}

import os as _os

ABLATE = int(_os.environ.get("KERNEL_ABLATE", "5"))


def _build(n_loop=1):
    from contextlib import ExitStack

    import concourse.bass as bass
    import concourse.tile as tile
    from concourse import bacc, mybir

    f32 = mybir.dt.float32
    bf16 = mybir.dt.bfloat16
    i32 = mybir.dt.int32
    Alu = mybir.AluOpType
    Act = mybir.ActivationFunctionType
    X = mybir.AxisListType.X

    nc = bacc.Bacc("TRN2", target_bir_lowering=False, debug=False)

    image_d = nc.dram_tensor("image", [BL, N, D], f32, kind="ExternalInput").ap()
    text_d = nc.dram_tensor("text", [BL, T, D], f32, kind="ExternalInput").ap()
    target_d = nc.dram_tensor("target", [BL, N, D], f32, kind="ExternalInput").ap()
    pm_d = nc.dram_tensor("pm", [BL, T], i32, kind="ExternalInput").ap()
    idf_d = nc.dram_tensor("idf", [128, 128], f32, kind="ExternalInput").ap()
    out_cols_d = nc.dram_tensor("out_cols", [128, 4], f32, kind="ExternalOutput").ap()

    with tile.TileContext(nc) as tc, ExitStack() as ctx:
        cp = ctx.enter_context(tc.tile_pool(name="const", bufs=1))
        ld = ctx.enter_context(tc.tile_pool(name="ld", bufs=4))
        ldi = ctx.enter_context(tc.tile_pool(name="ldi", bufs=3))
        xtp = ctx.enter_context(tc.tile_pool(name="xtp", bufs=3))
        tTp = ctx.enter_context(tc.tile_pool(name="tTp", bufs=3))
        xTp = ctx.enter_context(tc.tile_pool(name="xTp", bufs=2))
        rowp = ctx.enter_context(tc.tile_pool(name="rowp", bufs=3))
        colp = ctx.enter_context(tc.tile_pool(name="colp", bufs=3))
        sbk = ctx.enter_context(tc.tile_pool(name="sbk", bufs=3))
        dfp = ctx.enter_context(tc.tile_pool(name="dfp", bufs=2))
        kp = ctx.enter_context(tc.tile_pool(name="kp", bufs=1))
        psT = ctx.enter_context(
            tc.tile_pool(name="psT", bufs=3, space=bass.MemorySpace.PSUM)
        )
        psG = ctx.enter_context(
            tc.tile_pool(name="psG", bufs=2, space=bass.MemorySpace.PSUM)
        )
        psB = ctx.enter_context(
            tc.tile_pool(name="psB", bufs=1, space=bass.MemorySpace.PSUM)
        )
        psS = ctx.enter_context(
            tc.tile_pool(name="psS", bufs=2, space=bass.MemorySpace.PSUM)
        )

        # constants
        idf = cp.tile([128, 128], f32)
        nc.sync.dma_start(idf[:], idf_d[:])
        idbf = cp.tile([128, 128], bf16)
        nc.gpsimd.dma_start(idbf[:], idf_d[:])
        ones64 = cp.tile([1, 64], f32)
        nc.vector.memset(ones64[:], 1.0)
        tok_buf = cp.tile([128, PAIRS], f32)
        imgbuf = cp.tile([128, 2 * BL], f32)
        outc = cp.tile([128, 4], f32)

        target_flat = target_d.rearrange("b n d -> (b n) d")

        def transpose_batch(ps, dst_sb, a_tile, b_tile, c0, eng_copy):
            """PE-transpose chunks c0, c0+1 of (a,b) into psum then copy to
            dst_sb cols [416*(c0//2) : +416] as bf16."""
            for i, c in enumerate((c0, c0 + 1)):
                off = 208 * i
                nc.tensor.matmul(
                    ps[:, off : off + 128],
                    a_tile[:, 128 * c : 128 * (c + 1)],
                    idbf[:, :],
                    start=True,
                    stop=True,
                )
                nc.tensor.matmul(
                    ps[:, off + 128 : off + 208],
                    b_tile[0:80, 128 * c : 128 * (c + 1)],
                    idbf[0:80, 0:80],
                    start=True,
                    stop=True,
                )
            k = c0 // 2
            if eng_copy == "dve":
                nc.vector.tensor_copy(dst_sb[:, 416 * k : 416 * k + 416], ps[:, 0:416])
            else:
                nc.scalar.copy(dst_sb[:, 416 * k : 416 * k + 416], ps[:, 0:416])

        def body():
            nc.vector.memset(outc[:], 0.0)
            nc.vector.memset(imgbuf[:], 0.0)

            for p in range(PAIRS):
                xt = xtp.tile([128, D], bf16, tag="xt")
                tTs = []
                tsq_col = colp.tile([128, 1], f32, tag="tsq")
                for j in range(2):
                    b = 2 * p + j
                    # ---- cast loads (fp32 DRAM -> bf16 SBUF, SWDGE) ----
                    tgt_a = ld.tile([128, D], bf16, tag="tgt_a")
                    nc.gpsimd.dma_start(tgt_a[:], target_d[b, 0:128, :])
                    tgt_b = ld.tile([80, D], bf16, tag="tgt_b")
                    if b < BL - 1:
                        # pad rows 69:80 with neighbor-sample rows; they land in
                        # transposed columns 197:207 which are always sliced out
                        nc.gpsimd.dma_start(
                            tgt_b[:], target_flat[N * b + 128 : N * b + 208, :]
                        )
                    else:
                        nc.vector.memset(tgt_b[64:80, :], 0.0)
                        nc.gpsimd.dma_start(tgt_b[0:69, :], target_d[b, 128:197, :])
                    img_a = ldi.tile([128, D], bf16, tag="img_a")
                    nc.gpsimd.dma_start(img_a[:], image_d[b, 0:128, :])
                    img_b = ldi.tile([80, D], bf16, tag="img_b")
                    nc.gpsimd.dma_start(img_b[0:69, :], image_d[b, 128:197, :])
                    nc.gpsimd.dma_start(xt[64 * j : 64 * (j + 1), :], text_d[b, :, :])

                    # ---- target transpose via PE (3 batches of 2 chunks) ----
                    tT = tTp.tile([128, 6 * CW], bf16, tag="tT")
                    if ABLATE >= 2:
                        for k in range(3):
                            ps = psT.tile([128, 512], f32, tag="tp")
                            eng = "dve" if k == 0 else "act"
                            transpose_batch(ps, tT, tgt_a, tgt_b, 2 * k, eng)
                    tTs.append(tT)

                    if ABLATE < 3:
                        continue

                    # ---- rsq columns then psum row [1, 208] ----
                    rsqc0 = colp.tile([128, 1], f32, tag="rsqc0")
                    sqj0 = dfp.tile([128, D], bf16, tag="sqjunk0")
                    nc.vector.scalar_tensor_tensor(
                        sqj0[:],
                        tgt_a[:], 1.0, tgt_a[:],
                        op0=Alu.mult, op1=Alu.mult, accum_out=rsqc0[:],
                    )
                    rsqc1 = colp.tile([80, 1], f32, tag="rsqc1")
                    sqj1 = dfp.tile([80, D], bf16, tag="sqjunk1")
                    nc.vector.scalar_tensor_tensor(
                        sqj1[0:80, :],
                        tgt_b[0:80, :], 1.0, tgt_b[0:80, :],
                        op0=Alu.mult, op1=Alu.mult, accum_out=rsqc1[:],
                    )
                    rsq = psS.tile([1, CW], f32, tag="small")
                    nc.tensor.matmul(
                        rsq[0:1, 0:128], rsqc0[:], idf[:, :], start=True, stop=True
                    )
                    nc.tensor.matmul(
                        rsq[0:1, 128:208], rsqc1[:], idf[0:80, 0:80],
                        start=True, stop=True,
                    )
                    r_row = rowp.tile([1, CW], f32, tag="r_row")
                    nc.scalar.activation(r_row[:, 0:197], rsq[:, 0:197], Act.Sqrt)
                    rinv_row = rowp.tile([1, CW], f32, tag="rinv_row")
                    nc.vector.reciprocal(rinv_row[:, 0:197], r_row[:, 0:197])
                    rsqC_row = rowp.tile([1, CW], f32, tag="rsqC_row")
                    nc.scalar.activation(
                        rsqC_row[:, 0:197], rsq[:, 0:197], Act.Copy, bias=C_OFF
                    )

                    # ---- broadcasts into psum [128, 416]: rinv | rsqC ----
                    if j == 0:
                        bc = psB.tile([128, 2 * CW], f32, tag="bc")
                    nc.tensor.matmul(
                        bc[64 * j : 64 * (j + 1), 0:197],
                        ones64[:], rinv_row[:, 0:197], start=True, stop=True,
                    )
                    nc.tensor.matmul(
                        bc[64 * j : 64 * (j + 1), CW : CW + 197],
                        ones64[:], rsqC_row[:, 0:197], start=True, stop=True,
                    )

                    # ---- image loss ----
                    if ABLATE < 4:
                        continue
                    diff_a = dfp.tile([128, D], bf16, tag="diff_a")
                    nc.vector.tensor_tensor(diff_a[:], img_a[:], tgt_a[:], Alu.subtract)
                    diff_b = dfp.tile([80, D], bf16, tag="diff_b")
                    nc.gpsimd.tensor_tensor(
                        diff_b[0:69, :], img_b[0:69, :], tgt_b[0:69, :], Alu.subtract
                    )
                    # per-row sum of squares straight into imgbuf columns
                    dsqj0 = dfp.tile([128, D], bf16, tag="dsqjunk0")
                    nc.scalar.activation(
                        dsqj0[:],
                        diff_a[:], Act.Square,
                        accum_out=imgbuf[:, 2 * b : 2 * b + 1],
                    )
                    dsqj1 = dfp.tile([80, D], bf16, tag="dsqjunk1")
                    nc.vector.scalar_tensor_tensor(
                        dsqj1[0:69, :],
                        diff_b[0:69, :], 1.0, diff_b[0:69, :],
                        op0=Alu.mult, op1=Alu.mult,
                        accum_out=imgbuf[0:69, 2 * b + 1 : 2 * b + 2],
                    )

                if ABLATE < 5:
                    continue

                # ---- text transpose for the pair (2 psum batches) ----
                xT = xTp.tile([128, D], bf16, tag="xT")
                ps1 = psT.tile([128, 512], f32, tag="tp")
                for c in range(4):
                    nc.tensor.matmul(
                        ps1[:, 128 * c : 128 * (c + 1)],
                        xt[:, 128 * c : 128 * (c + 1)],
                        idbf[:, :], start=True, stop=True,
                    )
                nc.vector.tensor_copy(xT[:, 0:512], ps1[:, 0:512])
                ps2 = psT.tile([128, 512], f32, tag="tp")
                for c in range(4, 6):
                    nc.tensor.matmul(
                        ps2[:, 128 * (c - 4) : 128 * (c - 3)],
                        xt[:, 128 * c : 128 * (c + 1)],
                        idbf[:, :], start=True, stop=True,
                    )
                nc.scalar.activation(xT[:, 512:768], ps2[:, 0:256], Act.Copy)

                # textsq as a pair-stacked column
                sqxj = dfp.tile([128, D], bf16, tag="sqxjunk")
                nc.vector.scalar_tensor_tensor(
                    sqxj[:],
                    xt[:], 1.0, xt[:],
                    op0=Alu.mult, op1=Alu.mult, accum_out=tsq_col[:],
                )

                # ---- G = text . target (pair-stacked [128, 197] psum) ----
                G = psG.tile([128, CW], f32, tag="G")
                for j in range(2):
                    for c in range(6):
                        nc.tensor.matmul(
                            G[64 * j : 64 * (j + 1), 0:197],
                            xT[:, 128 * c + 64 * j : 128 * c + 64 * (j + 1)],
                            tTs[j][:, CW * c : CW * c + 197],
                            start=(c == 0),
                            stop=(c == 5),
                        )

                # ---- selection block ----
                G_sb = sbk.tile([128, CW], f32, tag="G_sb")
                nc.scalar.copy(G_sb[:, 0:197], G[:, 0:197])
                s = sbk.tile([128, CW], f32, tag="s")
                nc.vector.tensor_tensor(
                    s[:, 0:197], G_sb[:, 0:197], bc[:, 0:197], Alu.mult
                )
                m = sbk.tile([128, 1], f32, tag="m")
                nc.vector.tensor_reduce(m[:], s[:, 1:197], X, Alu.max)
                v = sbk.tile([128, CW], f32, tag="v")
                nc.vector.scalar_tensor_tensor(
                    v[:, 0:196], G_sb[:, 1:197], -2.0, bc[:, CW + 1 : CW + 197],
                    op0=Alu.mult, op1=Alu.add,
                )
                y = sbk.tile([128, CW], f32, tag="y")
                nc.vector.scalar_tensor_tensor(
                    y[:, 0:196], s[:, 1:197], m[:], v[:, 0:196],
                    op0=Alu.is_ge, op1=Alu.mult,
                )
                vsel = sbk.tile([128, 1], f32, tag="vsel")
                nc.vector.tensor_reduce(vsel[:], y[:, 0:196], X, Alu.max)

                # tok_sq column for this pair: textsq + (vsel - C)
                nc.vector.scalar_tensor_tensor(
                    tok_buf[:, p : p + 1], vsel[:], -C_OFF, tsq_col[:],
                    op0=Alu.add, op1=Alu.add,
                )

            # ---- keep mask ----
            if ABLATE < 5:
                nc.sync.dma_start(out_cols_d[:], outc[:])
                return
            pm_t = kp.tile([BL, T], i32, tag="pm_t")
            nc.sync.dma_start(pm_t[:], pm_d[:])
            pmf = kp.tile([BL, T], f32, tag="pmf")
            nc.vector.tensor_copy(pmf[:], pm_t[:])
            pmT = psS.tile([T, BL], f32, tag="small")
            nc.tensor.matmul(pmT[:], pmf[:], idf[0:16, 0:16], start=True, stop=True)
            kT = kp.tile([128, PAIRS], f32, tag="kT")
            pmT3 = pmT[:].rearrange("p (e two) -> p two e", two=2)
            nc.vector.tensor_copy(kT[0:64, :], pmT3[:, 0, :])
            nc.vector.tensor_copy(kT[64:128, :], pmT3[:, 1, :])
            keep = kp.tile([128, PAIRS], f32, tag="keep")
            nc.vector.tensor_scalar(keep[:], kT[:], 0.0, None, op0=Alu.is_equal)
            nc.vector.memset(keep[0:1, :], 0.0)
            nc.vector.memset(keep[64:65, :], 0.0)

            junk = kp.tile([128, PAIRS], f32, tag="junk")
            nc.vector.scalar_tensor_tensor(
                junk[:], tok_buf[:], 1.0, keep[:], op0=Alu.mult, op1=Alu.mult,
                accum_out=outc[:, 0:1],
            )
            nc.vector.tensor_reduce(outc[:, 1:2], keep[:], X, Alu.add)

            # ---- cls term ----
            tcls = kp.tile([BL, D], bf16, tag="tcls")
            nc.gpsimd.dma_start(tcls[:], text_d[:, 0, :])
            icls = kp.tile([BL, D], bf16, tag="icls")
            nc.gpsimd.dma_start(icls[:], image_d[:, 0, :])
            dcls = kp.tile([BL, D], bf16, tag="dcls")
            nc.vector.tensor_tensor(dcls[:], tcls[:], icls[:], Alu.subtract)
            cjunk = kp.tile([BL, D], f32, tag="cjunk")
            nc.vector.scalar_tensor_tensor(
                cjunk[:], dcls[:], 1.0, dcls[:], op0=Alu.mult, op1=Alu.mult,
                accum_out=outc[0:BL, 2:3],
            )

            # ---- image loss total per row ----
            nc.vector.tensor_reduce(outc[:, 3:4], imgbuf[:], X, Alu.add)

            nc.sync.dma_start(out_cols_d[:], outc[:])

        if n_loop > 1:
            with tc.For_i(0, n_loop, 1):
                body()
        else:
            body()

    nc.compile()
    return nc


def _get_nc(n_loop=1):
    if n_loop not in _CACHE:
        _CACHE[n_loop] = _build(n_loop)
    return _CACHE[n_loop]


def _run(nc, image, text, target, padding_mask):
    from concourse.bass_utils import run_bass_kernel_spmd

    image = np.ascontiguousarray(np.asarray(image, dtype=np.float32))
    text = np.ascontiguousarray(np.asarray(text, dtype=np.float32))
    target = np.ascontiguousarray(np.asarray(target, dtype=np.float32))
    pm = np.ascontiguousarray(np.asarray(padding_mask, dtype=np.int32))
    idf = np.eye(128, dtype=np.float32)

    in_maps = []
    for c in range(NCORES):
        sl = slice(c * BL, (c + 1) * BL)
        in_maps.append(
            {
                "image": image[sl],
                "text": text[sl],
                "target": target[sl],
                "pm": pm[sl],
                "idf": idf,
            }
        )
    res = run_bass_kernel_spmd(nc, in_maps, list(range(NCORES)))
    return res


def _combine(results):
    masked = 0.0
    keep = 0.0
    cls = 0.0
    img = 0.0
    for r in results:
        oc = r["out_cols"].astype(np.float64)
        masked += oc[:, 0].sum()
        keep += oc[:, 1].sum()
        cls += oc[0:BL, 2].sum()
        img += oc[:, 3].sum()
    kd_text = (cls + masked) / ((B + keep) * D)
    kd_img = img / (B * N * D)
    return np.asarray((kd_text + kd_img) / 2.0, dtype=np.float32)


def kernel(image, text, target, padding_mask):
    nc = _get_nc(1)
    res = _run(nc, image, text, target, padding_mask)
    return _combine(res.results)



# revision 2
# speedup vs baseline: 223.5963x; 223.5963x over previous
"""Trainium2 Bass kernel for nn_CMLITargetLoss (CMLI target loss).

Data parallel: batch 128 -> 16 samples per core x 8 cores. Inputs are cast
fp32->bf16 during DMA (SWDGE); all accumulation is fp32.

Per sample:
  - target^T via PE matmul-identity transposes (bf16 -> fp32 psum), copied
    back to SBUF as bf16 in [128,416] batches
  - rsq[n] = sum_d target^2 via scalar_tensor_tensor accum (per-row sums),
    then tiny PE matmuls to turn the columns into a [1,197] psum row
  - r = sqrt(rsq) on ACT, rinv = 1/r on DVE, rsqC = rsq + C on ACT;
    broadcast rows to [128,197] via ones-matmul (pair-stacked)
  - G[t,n] = text . target via PE matmul (bf16), two samples stacked
  - s = G * rinv; m = rowmax(s); mask = (s >= m); v = rsqC - 2G;
    vsel = max(mask*v) - C  =>  tok_sq = ||text_t||^2 + rsq[n*] - 2 G[t,n*]
  - image loss: diff = image - target (DVE/POOL), per-row sum of squares via
    ACT Square+accum / DVE stt accum into a [128,32] column buffer
Host combines the 8 cores' partial sums in float64.

Outputs per core: out_cols [128,4] f32: col0 masked tok_sq partials,
col1 keep partials, col2 rows 0:16 cls partials, col3 image-loss partials.
"""

import numpy as np

B, T, N, D = 128, 64, 197, 768
NCORES = 8
BL = B // NCORES  # 16 samples per core
PAIRS = BL // 2
C_OFF = float(2.0**20)
CW = 208  # transposed-target column block (128 + 80)

_CACHE = {}

import os as _os

ABLATE = int(_os.environ.get("KERNEL_ABLATE", "5"))


def _build(n_loop=1):
    from contextlib import ExitStack

    import concourse.bass as bass
    import concourse.tile as tile
    from concourse import bacc, mybir

    f32 = mybir.dt.float32
    bf16 = mybir.dt.bfloat16
    i32 = mybir.dt.int32
    Alu = mybir.AluOpType
    Act = mybir.ActivationFunctionType
    X = mybir.AxisListType.X

    nc = bacc.Bacc("TRN2", target_bir_lowering=False, debug=False)

    image_d = nc.dram_tensor("image", [BL, N, D], f32, kind="ExternalInput").ap()
    text_d = nc.dram_tensor("text", [BL, T, D], f32, kind="ExternalInput").ap()
    target_d = nc.dram_tensor("target", [BL, N, D], f32, kind="ExternalInput").ap()
    pm_d = nc.dram_tensor("pm", [BL, T], i32, kind="ExternalInput").ap()
    idf_d = nc.dram_tensor("idf", [128, 128], f32, kind="ExternalInput").ap()
    out_cols_d = nc.dram_tensor("out_cols", [128, 4], f32, kind="ExternalOutput").ap()

    with tile.TileContext(nc) as tc, ExitStack() as ctx:
        cp = ctx.enter_context(tc.tile_pool(name="const", bufs=1))
        ld = ctx.enter_context(tc.tile_pool(name="ld", bufs=4))
        ldi = ctx.enter_context(tc.tile_pool(name="ldi", bufs=3))
        xtp = ctx.enter_context(tc.tile_pool(name="xtp", bufs=3))
        tTp = ctx.enter_context(tc.tile_pool(name="tTp", bufs=3))
        xTp = ctx.enter_context(tc.tile_pool(name="xTp", bufs=2))
        rowp = ctx.enter_context(tc.tile_pool(name="rowp", bufs=3))
        colp = ctx.enter_context(tc.tile_pool(name="colp", bufs=3))
        sbk = ctx.enter_context(tc.tile_pool(name="sbk", bufs=3))
        dfp = ctx.enter_context(tc.tile_pool(name="dfp", bufs=2))
        kp = ctx.enter_context(tc.tile_pool(name="kp", bufs=1))
        psT = ctx.enter_context(
            tc.tile_pool(name="psT", bufs=3, space=bass.MemorySpace.PSUM)
        )
        psG = ctx.enter_context(
            tc.tile_pool(name="psG", bufs=2, space=bass.MemorySpace.PSUM)
        )
        psB = ctx.enter_context(
            tc.tile_pool(name="psB", bufs=1, space=bass.MemorySpace.PSUM)
        )
        psS = ctx.enter_context(
            tc.tile_pool(name="psS", bufs=2, space=bass.MemorySpace.PSUM)
        )

        # constants
        idf = cp.tile([128, 128], f32)
        nc.sync.dma_start(idf[:], idf_d[:])
        idbf = cp.tile([128, 128], bf16)
        nc.gpsimd.dma_start(idbf[:], idf_d[:])
        ones64 = cp.tile([1, 64], f32)
        nc.vector.memset(ones64[:], 1.0)
        tok_buf = cp.tile([128, PAIRS], f32)
        imgbuf = cp.tile([128, 2 * BL], f32)
        outc = cp.tile([128, 4], f32)

        target_flat = target_d.rearrange("b n d -> (b n) d")

        def transpose_batch(ps, dst_sb, a_tile, b_tile, c0, eng_copy):
            """PE-transpose chunks c0, c0+1 of (a,b) into psum then copy to
            dst_sb cols [416*(c0//2) : +416] as bf16."""
            for i, c in enumerate((c0, c0 + 1)):
                off = 208 * i
                nc.tensor.matmul(
                    ps[:, off : off + 128],
                    a_tile[:, 128 * c : 128 * (c + 1)],
                    idbf[:, :],
                    start=True,
                    stop=True,
                )
                nc.tensor.matmul(
                    ps[:, off + 128 : off + 208],
                    b_tile[0:80, 128 * c : 128 * (c + 1)],
                    idbf[0:80, 0:80],
                    start=True,
                    stop=True,
                )
            k = c0 // 2
            if eng_copy == "dve":
                nc.vector.tensor_copy(dst_sb[:, 416 * k : 416 * k + 416], ps[:, 0:416])
            else:
                nc.scalar.copy(dst_sb[:, 416 * k : 416 * k + 416], ps[:, 0:416])

        def body():
            nc.vector.memset(outc[:], 0.0)
            nc.vector.memset(imgbuf[:], 0.0)

            for p in range(PAIRS):
                xt = xtp.tile([128, D], bf16, tag="xt")
                tTs = []
                tsq_col = colp.tile([128, 1], f32, tag="tsq")
                for j in range(2):
                    b = 2 * p + j
                    # ---- cast loads (fp32 DRAM -> bf16 SBUF, SWDGE) ----
                    tgt_a = ld.tile([128, D], bf16, tag="tgt_a")
                    nc.gpsimd.dma_start(tgt_a[:], target_d[b, 0:128, :])
                    tgt_b = ld.tile([80, D], bf16, tag="tgt_b")
                    if b < BL - 1:
                        # pad rows 69:80 with neighbor-sample rows; they land in
                        # transposed columns 197:207 which are always sliced out
                        nc.gpsimd.dma_start(
                            tgt_b[:], target_flat[N * b + 128 : N * b + 208, :]
                        )
                    else:
                        nc.vector.memset(tgt_b[64:80, :], 0.0)
                        nc.gpsimd.dma_start(tgt_b[0:69, :], target_d[b, 128:197, :])
                    img_a = ldi.tile([128, D], bf16, tag="img_a")
                    nc.gpsimd.dma_start(img_a[:], image_d[b, 0:128, :])
                    img_b = ldi.tile([80, D], bf16, tag="img_b")
                    nc.gpsimd.dma_start(img_b[0:69, :], image_d[b, 128:197, :])
                    nc.gpsimd.dma_start(xt[64 * j : 64 * (j + 1), :], text_d[b, :, :])

                    # ---- target transpose via PE (3 batches of 2 chunks) ----
                    tT = tTp.tile([128, 6 * CW], bf16, tag="tT")
                    if ABLATE >= 2:
                        for k in range(3):
                            ps = psT.tile([128, 512], f32, tag="tp")
                            eng = "dve" if k == 0 else "act"
                            transpose_batch(ps, tT, tgt_a, tgt_b, 2 * k, eng)
                    tTs.append(tT)

                    if ABLATE < 3:
                        continue

                    # ---- rsq columns then psum row [1, 208] ----
                    rsqc0 = colp.tile([128, 1], f32, tag="rsqc0")
                    sqj0 = dfp.tile([128, D], bf16, tag="sqjunk0")
                    nc.vector.scalar_tensor_tensor(
                        sqj0[:],
                        tgt_a[:], 1.0, tgt_a[:],
                        op0=Alu.mult, op1=Alu.mult, accum_out=rsqc0[:],
                    )
                    rsqc1 = colp.tile([80, 1], f32, tag="rsqc1")
                    sqj1 = dfp.tile([80, D], bf16, tag="sqjunk1")
                    nc.vector.scalar_tensor_tensor(
                        sqj1[0:80, :],
                        tgt_b[0:80, :], 1.0, tgt_b[0:80, :],
                        op0=Alu.mult, op1=Alu.mult, accum_out=rsqc1[:],
                    )
                    rsq = psS.tile([1, CW], f32, tag="small")
                    nc.tensor.matmul(
                        rsq[0:1, 0:128], rsqc0[:], idf[:, :], start=True, stop=True
                    )
                    nc.tensor.matmul(
                        rsq[0:1, 128:208], rsqc1[:], idf[0:80, 0:80],
                        start=True, stop=True,
                    )
                    r_row = rowp.tile([1, CW], f32, tag="r_row")
                    nc.scalar.activation(r_row[:, 0:197], rsq[:, 0:197], Act.Sqrt)
                    rinv_row = rowp.tile([1, CW], f32, tag="rinv_row")
                    nc.vector.reciprocal(rinv_row[:, 0:197], r_row[:, 0:197])
                    rsqC_row = rowp.tile([1, CW], f32, tag="rsqC_row")
                    nc.scalar.activation(
                        rsqC_row[:, 0:197], rsq[:, 0:197], Act.Copy, bias=C_OFF
                    )

                    # ---- broadcasts into psum [128, 416]: rinv | rsqC ----
                    if j == 0:
                        bc = psB.tile([128, 2 * CW], f32, tag="bc")
                    nc.tensor.matmul(
                        bc[64 * j : 64 * (j + 1), 0:197],
                        ones64[:], rinv_row[:, 0:197], start=True, stop=True,
                    )
                    nc.tensor.matmul(
                        bc[64 * j : 64 * (j + 1), CW : CW + 197],
                        ones64[:], rsqC_row[:, 0:197], start=True, stop=True,
                    )

                    # ---- image loss ----
                    if ABLATE < 4:
                        continue
                    diff_a = dfp.tile([128, D], bf16, tag="diff_a")
                    nc.vector.tensor_tensor(diff_a[:], img_a[:], tgt_a[:], Alu.subtract)
                    diff_b = dfp.tile([80, D], bf16, tag="diff_b")
                    nc.gpsimd.tensor_tensor(
                        diff_b[0:69, :], img_b[0:69, :], tgt_b[0:69, :], Alu.subtract
                    )
                    # per-row sum of squares straight into imgbuf columns
                    dsqj0 = dfp.tile([128, D], bf16, tag="dsqjunk0")
                    nc.scalar.activation(
                        dsqj0[:],
                        diff_a[:], Act.Square,
                        accum_out=imgbuf[:, 2 * b : 2 * b + 1],
                    )
                    dsqj1 = dfp.tile([80, D], bf16, tag="dsqjunk1")
                    nc.vector.scalar_tensor_tensor(
                        dsqj1[0:69, :],
                        diff_b[0:69, :], 1.0, diff_b[0:69, :],
                        op0=Alu.mult, op1=Alu.mult,
                        accum_out=imgbuf[0:69, 2 * b + 1 : 2 * b + 2],
                    )

                if ABLATE < 5:
                    continue

                # ---- text transpose for the pair (2 psum batches) ----
                xT = xTp.tile([128, D], bf16, tag="xT")
                ps1 = psT.tile([128, 512], f32, tag="tp")
                for c in range(4):
                    nc.tensor.matmul(
                        ps1[:, 128 * c : 128 * (c + 1)],
                        xt[:, 128 * c : 128 * (c + 1)],
                        idbf[:, :], start=True, stop=True,
                    )
                nc.vector.tensor_copy(xT[:, 0:512], ps1[:, 0:512])
                ps2 = psT.tile([128, 512], f32, tag="tp")
                for c in range(4, 6):
                    nc.tensor.matmul(
                        ps2[:, 128 * (c - 4) : 128 * (c - 3)],
                        xt[:, 128 * c : 128 * (c + 1)],
                        idbf[:, :], start=True, stop=True,
                    )
                nc.scalar.activation(xT[:, 512:768], ps2[:, 0:256], Act.Copy)

                # textsq as a pair-stacked column
                sqxj = dfp.tile([128, D], bf16, tag="sqxjunk")
                nc.vector.scalar_tensor_tensor(
                    sqxj[:],
                    xt[:], 1.0, xt[:],
                    op0=Alu.mult, op1=Alu.mult, accum_out=tsq_col[:],
                )

                # ---- G = text . target (pair-stacked [128, 197] psum) ----
                G = psG.tile([128, CW], f32, tag="G")
                for j in range(2):
                    for c in range(6):
                        nc.tensor.matmul(
                            G[64 * j : 64 * (j + 1), 0:197],
                            xT[:, 128 * c + 64 * j : 128 * c + 64 * (j + 1)],
                            tTs[j][:, CW * c : CW * c + 197],
                            start=(c == 0),
                            stop=(c == 5),
                        )

                # ---- selection block ----
                G_sb = sbk.tile([128, CW], f32, tag="G_sb")
                nc.scalar.copy(G_sb[:, 0:197], G[:, 0:197])
                s = sbk.tile([128, CW], f32, tag="s")
                nc.vector.tensor_tensor(
                    s[:, 0:197], G_sb[:, 0:197], bc[:, 0:197], Alu.mult
                )
                m = sbk.tile([128, 1], f32, tag="m")
                nc.vector.tensor_reduce(m[:], s[:, 1:197], X, Alu.max)
                v = sbk.tile([128, CW], f32, tag="v")
                nc.vector.scalar_tensor_tensor(
                    v[:, 0:196], G_sb[:, 1:197], -2.0, bc[:, CW + 1 : CW + 197],
                    op0=Alu.mult, op1=Alu.add,
                )
                y = sbk.tile([128, CW], f32, tag="y")
                nc.vector.scalar_tensor_tensor(
                    y[:, 0:196], s[:, 1:197], m[:], v[:, 0:196],
                    op0=Alu.is_ge, op1=Alu.mult,
                )
                vsel = sbk.tile([128, 1], f32, tag="vsel")
                nc.vector.tensor_reduce(vsel[:], y[:, 0:196], X, Alu.max)

                # tok_sq column for this pair: textsq + (vsel - C)
                nc.vector.scalar_tensor_tensor(
                    tok_buf[:, p : p + 1], vsel[:], -C_OFF, tsq_col[:],
                    op0=Alu.add, op1=Alu.add,
                )

            # ---- keep mask ----
            if ABLATE < 5:
                nc.sync.dma_start(out_cols_d[:], outc[:])
                return
            pm_t = kp.tile([BL, T], i32, tag="pm_t")
            nc.sync.dma_start(pm_t[:], pm_d[:])
            pmf = kp.tile([BL, T], f32, tag="pmf")
            nc.vector.tensor_copy(pmf[:], pm_t[:])
            pmT = psS.tile([T, BL], f32, tag="small")
            nc.tensor.matmul(pmT[:], pmf[:], idf[0:16, 0:16], start=True, stop=True)
            kT = kp.tile([128, PAIRS], f32, tag="kT")
            pmT3 = pmT[:].rearrange("p (e two) -> p two e", two=2)
            nc.vector.tensor_copy(kT[0:64, :], pmT3[:, 0, :])
            nc.vector.tensor_copy(kT[64:128, :], pmT3[:, 1, :])
            keep = kp.tile([128, PAIRS], f32, tag="keep")
            nc.vector.tensor_scalar(keep[:], kT[:], 0.0, None, op0=Alu.is_equal)
            nc.vector.memset(keep[0:1, :], 0.0)
            nc.vector.memset(keep[64:65, :], 0.0)

            junk = kp.tile([128, PAIRS], f32, tag="junk")
            nc.vector.scalar_tensor_tensor(
                junk[:], tok_buf[:], 1.0, keep[:], op0=Alu.mult, op1=Alu.mult,
                accum_out=outc[:, 0:1],
            )
            nc.vector.tensor_reduce(outc[:, 1:2], keep[:], X, Alu.add)

            # ---- cls term ----
            tcls = kp.tile([BL, D], bf16, tag="tcls")
            nc.gpsimd.dma_start(tcls[:], text_d[:, 0, :])
            icls = kp.tile([BL, D], bf16, tag="icls")
            nc.gpsimd.dma_start(icls[:], image_d[:, 0, :])
            dcls = kp.tile([BL, D], bf16, tag="dcls")
            nc.vector.tensor_tensor(dcls[:], tcls[:], icls[:], Alu.subtract)
            cjunk = kp.tile([BL, D], f32, tag="cjunk")
            nc.vector.scalar_tensor_tensor(
                cjunk[:], dcls[:], 1.0, dcls[:], op0=Alu.mult, op1=Alu.mult,
                accum_out=outc[0:BL, 2:3],
            )

            # ---- image loss total per row ----
            nc.vector.tensor_reduce(outc[:, 3:4], imgbuf[:], X, Alu.add)

            nc.sync.dma_start(out_cols_d[:], outc[:])

        if n_loop > 1:
            with tc.For_i(0, n_loop, 1):
                body()
        else:
            body()

    nc.compile()
    return nc


def _get_nc(n_loop=1):
    if n_loop not in _CACHE:
        _CACHE[n_loop] = _build(n_loop)
    return _CACHE[n_loop]


def _run(nc, image, text, target, padding_mask, **kw):
    from concourse.bass_utils import run_bass_kernel_spmd

    image = np.ascontiguousarray(np.asarray(image, dtype=np.float32))
    text = np.ascontiguousarray(np.asarray(text, dtype=np.float32))
    target = np.ascontiguousarray(np.asarray(target, dtype=np.float32))
    pm = np.ascontiguousarray(np.asarray(padding_mask, dtype=np.int32))
    idf = np.eye(128, dtype=np.float32)

    in_maps = []
    for c in range(NCORES):
        sl = slice(c * BL, (c + 1) * BL)
        in_maps.append(
            {
                "image": image[sl],
                "text": text[sl],
                "target": target[sl],
                "pm": pm[sl],
                "idf": idf,
            }
        )
    res = run_bass_kernel_spmd(nc, in_maps, list(range(NCORES)), **kw)
    return res


def _combine(results):
    masked = 0.0
    keep = 0.0
    cls = 0.0
    img = 0.0
    for r in results:
        oc = r["out_cols"].astype(np.float64)
        masked += oc[:, 0].sum()
        keep += oc[:, 1].sum()
        cls += oc[0:BL, 2].sum()
        img += oc[:, 3].sum()
    kd_text = (cls + masked) / ((B + keep) * D)
    kd_img = img / (B * N * D)
    return np.asarray((kd_text + kd_img) / 2.0, dtype=np.float32)


def kernel(image, text, target, padding_mask):
    nc = _get_nc(1)
    res = _run(nc, image, text, target, padding_mask)
    return _combine(res.results)

